# revision 9
# baseline (speedup 1.0000x reference)
"""RWKV-5 style block (nn_Block_14328010899778) on 8 trn2 NeuronCores.

Data-parallel over batch B=8 (one batch element per core). Activations are
feature-major ([C, T], features on partitions): every `act @ W.T` matmul uses
host-pre-transposed weights as lhsT and the activation as rhs. Matmuls run in
float32r (full PE rate, ~1e-4 rounding); the FFN runs in bf16.

Phases per core:
  P0: LN1 + token-shift lerps (xk/xv/xr) -> DRAM
  P1: projections rT, kT, v_tok + PE-transpose of kT into wk-scaled k_tok
  P2: chunked WKV (4 x 512): scoresT, decay mask, y, carried state, GroupNorm
  P3a: Wo + residual -> x1T;  P3b: LN2 + channel-mix lerps
  P4: FFN k = relu(xk@Wk_f.T)^2 (bf16) -> DRAM
  P5: FFN kv + sigmoid gate + residual -> x2T (output)
"""
import sys
for _p in ('/opt/trn_rl_repo', '/root/.axon_site/_ro/trn_rl_repo'):
    if _p not in sys.path:
        sys.path.insert(0, _p)

import numpy as np
import ml_dtypes

import concourse.bass as bass
import concourse.mybir as mybir
import concourse.tile as tile
from concourse import bacc
from concourse.bass_utils import run_bass_kernel_spmd

B, TT, C, H, S, F, CH = 8, 2048, 2048, 32, 64, 7168, 512
NB = C // 128          # 16 feature blocks
NCH = TT // CH         # 4 chunks
NPAIR = H // 2         # 16 head pairs
NFB = F // 128         # 56 ffn blocks
TH = TT // 2
EPS = 1e-5

f32 = mybir.dt.float32
f32r = mybir.dt.float32r
bf16 = mybir.dt.bfloat16
FP = mybir.ActivationFunctionType
OP = mybir.AluOpType


def build_kernel():
    nc = bacc.Bacc("TRN2", target_bir_lowering=False, debug=False)

    # ---------------- DRAM I/O ----------------
    xT = nc.dram_tensor("xT", [C, TT], f32r, kind="ExternalInput")
    tmshift = nc.dram_tensor("tmshift", [128, NB], f32, kind="ExternalInput")
    cmshift = nc.dram_tensor("cmshift", [128, NB], f32, kind="ExternalInput")
    s0 = nc.dram_tensor("s0", [128, NPAIR, S], f32r, kind="ExternalInput")

    WrT = nc.dram_tensor("WrT", [C, C], f32r, kind="ExternalInput")
    WkT = nc.dram_tensor("WkT", [C, C], f32r, kind="ExternalInput")
    WvT = nc.dram_tensor("WvT", [C, C], f32r, kind="ExternalInput")
    WoT = nc.dram_tensor("WoT", [C, C], f32r, kind="ExternalInput")
    WkfT = nc.dram_tensor("WkfT", [C, F], bf16, kind="ExternalInput")
    WvfT = nc.dram_tensor("WvfT", [F, C], bf16, kind="ExternalInput")
    WrfT = nc.dram_tensor("WrfT", [C, C], bf16, kind="ExternalInput")

    wmatT = nc.dram_tensor("wmatT", [H, CH, CH], f32r, kind="ExternalInput")
    wktok = nc.dram_tensor("wktok", [128, NB, H], f32, kind="ExternalInput")
    wbT = nc.dram_tensor("wbT", [2, NPAIR, TT], f32r, kind="ExternalInput")
    wspair = nc.dram_tensor("wspair", [128, NPAIR], f32, kind="ExternalInput")

    coefs = {}
    for nm in ("tmk", "tmv", "tmr", "cmk", "cmr", "ln1g", "ln1b", "ln2g",
               "ln2b", "gng", "gnb"):
        coefs[nm] = nc.dram_tensor(nm, [128, NB], f32, kind="ExternalInput")

    mask2 = nc.dram_tensor("mask2", [2, 128], f32r, kind="ExternalInput")
    seg2 = nc.dram_tensor("seg2", [128, 2], f32r, kind="ExternalInput")
    ones_col = nc.dram_tensor("ones_col", [128, 1], f32r, kind="ExternalInput")
    ones_row = nc.dram_tensor("ones_row", [1, 128], f32r, kind="ExternalInput")
    ident = nc.dram_tensor("ident", [128, 128], f32r, kind="ExternalInput")

    out_x2T = nc.dram_tensor("out_x2T", [C, TT], f32, kind="ExternalOutput")
    out_tmsh = nc.dram_tensor("out_tmsh", [128, NB], f32, kind="ExternalOutput")
    out_cmsh = nc.dram_tensor("out_cmsh", [128, NB], f32, kind="ExternalOutput")
    out_wkv = nc.dram_tensor("out_wkv", [128, NPAIR, S], f32, kind="ExternalOutput")

    # ---------------- DRAM scratch ----------------
    xkT = nc.dram_tensor("sc_xkT", [C, TT], f32r)
    xvT = nc.dram_tensor("sc_xvT", [C, TT], f32r)
    xrT = nc.dram_tensor("sc_xrT", [C, TT], f32r)
    rT = nc.dram_tensor("sc_rT", [C, TT], f32r)
    kTd = nc.dram_tensor("sc_kT", [C, TT], f32r)
    vtok = nc.dram_tensor("sc_vtok", [NPAIR, TT, 128], f32r)
    ktw = nc.dram_tensor("sc_ktw", [NPAIR, TT, 128], f32r)
    ynT = nc.dram_tensor("sc_ynT", [C, TT], f32r)
    x1T = nc.dram_tensor("sc_x1T", [C, TT], f32r)
    xkfT = nc.dram_tensor("sc_xkfT", [C, TT], bf16)
    xrfT = nc.dram_tensor("sc_xrfT", [C, TT], bf16)
    ktf = nc.dram_tensor("sc_ktf", [F, TT], bf16)

    def fm(dr):  # feature-major DRAM view: [C, TT] -> [128, NB, TT]
        return dr.rearrange("(b p) t -> p b t", p=128)

    with tile.TileContext(nc) as tc:
        with tc.tile_pool(name="singles", bufs=1) as singles:
            cf = {}
            for nm, dr in coefs.items():
                cf[nm] = singles.tile([128, NB], f32, tag=f"cf_{nm}", name=f"cf_{nm}")
                nc.sync.dma_start(cf[nm][:], dr[:])
            tmsh_sb = singles.tile([128, NB], f32, tag="tmsh")
            nc.sync.dma_start(tmsh_sb[:], tmshift[:])
            cmsh_sb = singles.tile([128, NB], f32, tag="cmsh")
            nc.sync.dma_start(cmsh_sb[:], cmshift[:])
            mask2_sb = singles.tile([2, 128], f32r, tag="mask2")
            nc.sync.dma_start(mask2_sb[:], mask2[:])
            seg2_sb = singles.tile([128, 2], f32r, tag="seg2")
            nc.sync.dma_start(seg2_sb[:], seg2[:])
            onesc_sb = singles.tile([128, 1], f32r, tag="onesc")
            nc.sync.dma_start(onesc_sb[:], ones_col[:])
            onesr_sb = singles.tile([1, 128], f32r, tag="onesr")
            nc.sync.dma_start(onesr_sb[:], ones_row[:])
            ident_sb = singles.tile([128, 128], f32r, tag="ident")
            nc.sync.dma_start(ident_sb[:], ident[:])
            wktok_sb = singles.tile([128, NB, H], f32, tag="wktok")
            nc.sync.dma_start(wktok_sb[:], wktok[:])
            wsp_sb = singles.tile([128, NPAIR], f32, tag="wsp")
            nc.sync.dma_start(wsp_sb[:], wspair[:])
            eps_sb = singles.tile([128, 1], f32, tag="eps")
            nc.vector.memset(eps_sb[:], EPS)

            # ---- helpers ----
            def layernorm(pool, pp, src_tile, g, b, dst_tile):
                """dst = LN(src) over C. src/dst: [128, NB, n]."""
                n = src_tile.shape[2]
                sq = pool.tile([128, NB, n], f32r, tag="ln_sq")
                nc.vector.tensor_mul(sq[:], src_tile[:], src_tile[:])
                psum_s = pp.tile([1, n], f32, tag="ln_s")
                psum_q = pp.tile([1, n], f32, tag="ln_q")
                for bi in range(NB):
                    nc.tensor.matmul(psum_s[:], onesc_sb[:], src_tile[:, bi, :],
                                     start=(bi == 0), stop=(bi == NB - 1))
                for bi in range(NB):
                    nc.tensor.matmul(psum_q[:], onesc_sb[:], sq[:, bi, :],
                                     start=(bi == 0), stop=(bi == NB - 1))
                st = pool.tile([1, 2, n], f32r, tag="ln_st")
                m, q = st[:, 0, :], st[:, 1, :]
                nc.vector.tensor_scalar_mul(m, psum_s[:], 1.0 / C)
                nc.vector.tensor_scalar_mul(q, psum_q[:], 1.0 / C)
                v = pool.tile([1, n], f32r, tag="ln_v")
                nc.vector.tensor_mul(v[:], m, m)
                nc.vector.tensor_sub(v[:], q, v[:])
                nc.scalar.activation(v[:], v[:], FP.Sqrt, bias=eps_sb[:1, :])
                with nc.allow_low_precision(reason="f32r rstd for broadcast matmul"):
                    nc.vector.reciprocal(v[:], v[:])
                pm = pp.tile([128, n], f32, tag="ln_bm")
                pr = pp.tile([128, n], f32, tag="ln_br")
                nc.tensor.matmul(pm[:], onesr_sb[:], m, start=True, stop=True)
                nc.tensor.matmul(pr[:], onesr_sb[:], v[:], start=True, stop=True)
                for bi in range(NB):
                    t0 = pool.tile([128, n], f32r, tag="ln_t0")
                    nc.vector.tensor_sub(t0[:], src_tile[:, bi, :], pm[:])
                    nc.vector.tensor_mul(t0[:], t0[:], pr[:])
                    nc.vector.tensor_scalar(dst_tile[:, bi, :], t0[:],
                                            g[:, bi:bi + 1], b[:, bi:bi + 1],
                                            op0=OP.mult, op1=OP.add)

            def lerp_out(ln_tile, xx0, coef, dst, dram, ch):
                """dst = xx + coef*(ln - xx), xx = shift(ln); DMA to dram chunk."""
                nc.vector.tensor_sub(dst[:, :, 1:], ln_tile[:, :, 1:],
                                     ln_tile[:, :, :CH - 1])
                nc.vector.tensor_sub(dst[:, :, 0:1], ln_tile[:, :, 0:1], xx0)
                for bi in range(NB):
                    nc.vector.tensor_scalar_mul(dst[:, bi, :], dst[:, bi, :],
                                                coef[:, bi:bi + 1])
                nc.vector.tensor_add(dst[:, :, 1:], dst[:, :, 1:],
                                     ln_tile[:, :, :CH - 1])
                nc.vector.tensor_add(dst[:, :, 0:1], dst[:, :, 0:1], xx0)
                nc.sync.dma_start(fm(dram)[:, :, ch * CH:(ch + 1) * CH], dst[:])

            # =========================================================
            # P0: LN1 + time-shift lerps -> xkT/xvT/xrT
            # =========================================================
            with tc.tile_pool(name="p0", bufs=1) as p0, \
                 tc.tile_pool(name="p0lo", bufs=2) as p0lo, \
                 tc.tile_pool(name="p0p", bufs=1, space="PSUM") as p0p, \
                 tc.tile_pool(name="p0c", bufs=2) as p0c:
                carry = None
                for ch in range(NCH):
                    csl = slice(ch * CH, (ch + 1) * CH)
                    xt = p0.tile([128, NB, CH], f32r, tag="x_in")
                    nc.sync.dma_start(xt[:], fm(xT)[:, :, csl])
                    ln = p0.tile([128, NB, CH], f32r, tag="ln1")
                    layernorm(p0, p0p, xt, cf["ln1g"], cf["ln1b"], ln)
                    xx0 = tmsh_sb[:, :, None] if ch == 0 else carry
                    for nm, dram in (("tmk", xkT), ("tmv", xvT), ("tmr", xrT)):
                        ot = p0lo.tile([128, NB, CH], f32r, tag="lo")
                        lerp_out(ln, xx0, cf[nm], ot, dram, ch)
                    ncar = p0c.tile([128, NB, 1], f32r, tag="carry")
                    nc.vector.tensor_copy(ncar[:], ln[:, :, CH - 1:CH])
                    carry = ncar
                    if ch == NCH - 1:
                        tms = p0c.tile([128, NB, 1], f32, tag="tms")
                        nc.vector.tensor_copy(tms[:], ln[:, :, CH - 1:CH])
                        nc.sync.dma_start(out_tmsh[:, :, None], tms[:])

            # =========================================================
            # P1: projections rT, kT (+ktw via PE transpose), v_tok
            # =========================================================
            with tc.tile_pool(name="p1a", bufs=1) as p1a, \
                 tc.tile_pool(name="p1w", bufs=4) as p1w, \
                 tc.tile_pool(name="p1wv", bufs=2) as p1wv, \
                 tc.tile_pool(name="p1o", bufs=4) as p1o, \
                 tc.tile_pool(name="p1p", bufs=2, space="PSUM") as p1p, \
                 tc.tile_pool(name="p1tp", bufs=2, space="PSUM") as p1tp:
                for half in range(2):
                    tsl = slice(half * TH, (half + 1) * TH)
                    for wsrc, asrc, odram, is_k in ((WrT, xrT, rT, False),
                                                    (WkT, xkT, kTd, True)):
                        act = p1a.tile([128, NB, TH], f32r, tag="act")
                        nc.sync.dma_start(act[:], fm(asrc)[:, :, tsl])
                        for co in range(NB):
                            ps = [p1p.tile([128, CH], f32, tag=f"pp{i}", name=f"pp{i}")
                                  for i in range(2)]
                            for ci in range(NB):
                                wblk = p1w.tile([128, 128], f32r, tag="wblk")
                                nc.sync.dma_start(
                                    wblk[:], wsrc[ci * 128:(ci + 1) * 128,
                                                  co * 128:(co + 1) * 128])
                                for t2 in range(2):
                                    nc.tensor.matmul(
                                        ps[t2][:], wblk[:],
                                        act[:, ci, t2 * CH:(t2 + 1) * CH],
                                        start=(ci == 0), stop=(ci == NB - 1))
                            for t2 in range(2):
                                ob = p1o.tile([128, CH], f32r, tag="ob")
                                nc.any.tensor_copy(ob[:], ps[t2][:])
                                nc.sync.dma_start(
                                    odram[co * 128:(co + 1) * 128,
                                          half * TH + t2 * CH:
                                          half * TH + (t2 + 1) * CH],
                                    ob[:])
                                if is_k:
                                    for tb in range(4):
                                        tp = p1tp.tile([128, 128], f32r, tag="tp")
                                        nc.tensor.transpose(
                                            tp[:], ob[:, tb * 128:(tb + 1) * 128],
                                            ident_sb[:])
                                        kw = p1o.tile([128, 2, 64], f32r, tag="kw")
                                        gt = half * 8 + t2 * 4 + tb
                                        nc.vector.tensor_tensor(
                                            kw[:],
                                            tp[:].rearrange("p (j u) -> p j u", j=2),
                                            wktok_sb[:, gt, 2 * co:2 * co + 2, None]
                                            .to_broadcast([128, 2, 64]),
                                            op=OP.mult)
                                        nc.sync.dma_start(
                                            ktw[co, gt * 128:(gt + 1) * 128, :],
                                            kw[:].rearrange("p j u -> p (j u)"))
                    # --- v_tok ---
                    act = p1a.tile([128, NB, TH], f32r, tag="act")
                    nc.sync.dma_start(act[:], fm(xvT)[:, :, tsl])
                    for co4 in range(4):
                        wv4 = p1wv.tile([128, NB, CH], f32r, tag="wv4")
                        nc.sync.dma_start(
                            wv4[:], WvT.rearrange("(cb p) c -> p cb c", p=128)
                            [:, :, co4 * CH:(co4 + 1) * CH])
                        for tb in range(8):
                            ps = p1p.tile([128, CH], f32, tag="pp0")
                            for ci in range(NB):
                                nc.tensor.matmul(
                                    ps[:], act[:, ci, tb * 128:(tb + 1) * 128],
                                    wv4[:, ci, :],
                                    start=(ci == 0), stop=(ci == NB - 1))
                            ob = p1o.tile([128, CH], f32r, tag="ob")
                            nc.any.tensor_copy(ob[:], ps[:])
                            gt = half * 8 + tb
                            nc.sync.dma_start(
                                vtok[4 * co4:4 * co4 + 4,
                                     gt * 128:(gt + 1) * 128, :]
                                .rearrange("j t c -> t j c"),
                                ob[:].rearrange("p (j c) -> p j c", j=4))

            # =========================================================
            # P2: chunked WKV -> ynT
            # =========================================================
            with tc.tile_pool(name="p2st", bufs=2) as p2st, \
                 tc.tile_pool(name="p2", bufs=2) as p2, \
                 tc.tile_pool(name="p2w", bufs=2) as p2w, \
                 tc.tile_pool(name="p2y", bufs=1) as p2y, \
                 tc.tile_pool(name="p2sc", bufs=1, space="PSUM") as p2sc, \
                 tc.tile_pool(name="p2p", bufs=1, space="PSUM") as p2p:
                s_cur = p2st.tile([128, NPAIR, S], f32r, tag="state")
                nc.sync.dma_start(s_cur[:], s0[:])
                for ch in range(NCH):
                    tsl = slice(ch * CH, (ch + 1) * CH)
                    wb_sb = p2.tile([2, NPAIR, CH], f32r, tag="wb")
                    nc.sync.dma_start(wb_sb[:], wbT[:, :, tsl])
                    yn_all = p2y.tile([128, NB, CH], f32r, tag="yn")
                    s_next = p2st.tile([128, NPAIR, S], f32r, tag="state")
                    for j in range(NPAIR):
                        rp = p2.tile([128, CH], f32r, tag="rp")
                        nc.sync.dma_start(rp[:], rT[j * 128:(j + 1) * 128, tsl])
                        kp = p2.tile([128, CH], f32r, tag="kp")
                        nc.sync.dma_start(kp[:], kTd[j * 128:(j + 1) * 128, tsl])
                        vp = p2.tile([128, 4, 128], f32r, tag="vp")
                        nc.sync.dma_start(
                            vp[:], vtok[j, tsl, :]
                            .rearrange("(tb p) c -> p tb c", p=128))
                        kwp = p2.tile([128, 4, 128], f32r, tag="kwp")
                        nc.sync.dma_start(
                            kwp[:], ktw[j, tsl, :]
                            .rearrange("(tb p) c -> p tb c", p=128))
                        # r_w = r * wb (broadcast over the 64 feats per head)
                        wbb = p2p.tile([128, CH], f32, tag="bcA")
                        nc.tensor.matmul(wbb[:], mask2_sb[:],
                                         wb_sb[:, j, :],
                                         start=True, stop=True)
                        rw = p2.tile([128, CH], f32r, tag="rw")
                        nc.vector.tensor_mul(rw[:], rp[:], wbb[:])
                        st0 = p2.tile([128, S], f32r, tag="st0")
                        nc.vector.tensor_scalar_mul(st0[:], s_cur[:, j, :],
                                                    wsp_sb[:, j:j + 1])
                        ysb = p2.tile([128, CH], f32r, tag="ysb")
                        for hl in range(2):
                            h = 2 * j + hl
                            hsl = slice(hl * 64, (hl + 1) * 64)
                            wmh = p2w.tile([128, 4, CH], f32r, tag="wmh")
                            nc.sync.dma_start(
                                wmh[:], wmatT[h]
                                .rearrange("(tb p) i -> p tb i", p=128))
                            scw = p2.tile([128, 4, CH], f32r, tag="scw")
                            for tbh in range(2):
                                sc = p2sc.tile([128, 2, CH], f32, tag="sc")
                                for t2 in range(2):
                                    tb = tbh * 2 + t2
                                    nc.tensor.matmul(
                                        sc[:, t2, :],
                                        kp[hsl, tb * 128:(tb + 1) * 128],
                                        rp[hsl, :], start=True, stop=True)
                                for t2 in range(2):
                                    tb = tbh * 2 + t2
                                    nc.vector.tensor_mul(
                                        scw[:, tb, :], sc[:, t2, :],
                                        wmh[:, tb, :])
                            yh = p2p.tile([64, CH], f32, tag="yh")
                            for tb in range(4):
                                nc.tensor.matmul(yh[:], vp[:, tb, hsl],
                                                 scw[:, tb, :],
                                                 start=(tb == 0), stop=False)
                            nc.tensor.matmul(yh[:], s_cur[hsl, j, :], rw[hsl, :],
                                             start=False, stop=True)
                            nc.any.tensor_copy(ysb[hsl, :], yh[:])
                            suh = p2p.tile([64, S], f32, tag="suh")
                            for tb in range(4):
                                nc.tensor.matmul(suh[:], kwp[:, tb, hsl],
                                                 vp[:, tb, hsl],
                                                 start=(tb == 0), stop=(tb == 3))
                            nc.vector.tensor_add(s_next[hsl, j, :], st0[hsl, :],
                                                 suh[:])
                        # GroupNorm over 64 feats per head
                        y2 = p2.tile([128, CH], f32r, tag="y2")
                        nc.vector.tensor_mul(y2[:], ysb[:], ysb[:])
                        pst1 = p2p.tile([2, CH], f32, tag="yh")
                        nc.tensor.matmul(pst1[:], seg2_sb[:], ysb[:],
                                         start=True, stop=True)
                        pst2 = p2p.tile([2, CH], f32, tag="suh")
                        nc.tensor.matmul(pst2[:], seg2_sb[:], y2[:],
                                         start=True, stop=True)
                        mq = p2.tile([2, 2, CH], f32r, tag="mq")
                        m, q = mq[:, 0, :], mq[:, 1, :]
                        nc.vector.tensor_scalar_mul(m, pst1[:], 1.0 / S)
                        nc.vector.tensor_scalar_mul(q, pst2[:], 1.0 / S)
                        vv = p2.tile([2, CH], f32r, tag="vv")
                        nc.vector.tensor_mul(vv[:], m, m)
                        nc.vector.tensor_sub(vv[:], q, vv[:])
                        nc.scalar.activation(vv[:], vv[:], FP.Sqrt, bias=eps_sb[:2, :])
                        with nc.allow_low_precision(reason="f32r rstd for broadcast matmul"):
                            nc.vector.reciprocal(vv[:], vv[:])
                        pmb = p2p.tile([128, CH], f32, tag="bcA")
                        nc.tensor.matmul(pmb[:], mask2_sb[:], m,
                                         start=True, stop=True)
                        prb = p2p.tile([128, CH], f32, tag="bcB")
                        nc.tensor.matmul(prb[:], mask2_sb[:], vv[:],
                                         start=True, stop=True)
                        t0 = p2.tile([128, CH], f32r, tag="gn_t0")
                        nc.vector.tensor_sub(t0[:], ysb[:], pmb[:])
                        nc.vector.tensor_mul(t0[:], t0[:], prb[:])
                        nc.vector.tensor_scalar(yn_all[:, j, :], t0[:],
                                                cf["gng"][:, j:j + 1],
                                                cf["gnb"][:, j:j + 1],
                                                op0=OP.mult, op1=OP.add)
                    nc.sync.dma_start(fm(ynT)[:, :, tsl], yn_all[:])
                    s_cur = s_next
                ow = p2.tile([128, NPAIR, S], f32, tag="ow")
                nc.vector.tensor_copy(ow[:], s_cur[:])
                nc.sync.dma_start(out_wkv[:], ow[:])

            # =========================================================
            # P3a: Wo + residual -> x1T
            # =========================================================
            with tc.tile_pool(name="p3a", bufs=1) as p3a, \
                 tc.tile_pool(name="p3w", bufs=4) as p3w, \
                 tc.tile_pool(name="p3o", bufs=4) as p3o, \
                 tc.tile_pool(name="p3p", bufs=2, space="PSUM") as p3p:
                for half in range(2):
                    tsl = slice(half * TH, (half + 1) * TH)
                    yna = p3a.tile([128, NB, TH], f32r, tag="yna")
                    nc.sync.dma_start(yna[:], fm(ynT)[:, :, tsl])
                    for co in range(NB):
                        ps = [p3p.tile([128, CH], f32, tag=f"po{i}", name=f"po{i}")
                              for i in range(2)]
                        for ci in range(NB):
                            wblk = p3w.tile([128, 128], f32r, tag="woblk")
                            nc.sync.dma_start(
                                wblk[:], WoT[ci * 128:(ci + 1) * 128,
                                             co * 128:(co + 1) * 128])
                            for t2 in range(2):
                                nc.tensor.matmul(
                                    ps[t2][:], wblk[:],
                                    yna[:, ci, t2 * CH:(t2 + 1) * CH],
                                    start=(ci == 0), stop=(ci == NB - 1))
                        for t2 in range(2):
                            xres = p3w.tile([128, CH], f32r, tag="xres")
                            nc.sync.dma_start(
                                xres[:], xT[co * 128:(co + 1) * 128,
                                            half * TH + t2 * CH:
                                            half * TH + (t2 + 1) * CH])
                            x1b = p3o.tile([128, CH], f32r, tag="x1b")
                            nc.vector.tensor_add(x1b[:], ps[t2][:], xres[:])
                            nc.sync.dma_start(
                                x1T[co * 128:(co + 1) * 128,
                                    half * TH + t2 * CH:
                                    half * TH + (t2 + 1) * CH],
                                x1b[:])

            # =========================================================
            # P3b: LN2 + channel-mix lerps -> xkfT/xrfT (bf16)
            # =========================================================
            with tc.tile_pool(name="p3b", bufs=1) as p3b, \
                 tc.tile_pool(name="p3blo", bufs=2) as p3blo, \
                 tc.tile_pool(name="p3bp", bufs=1, space="PSUM") as p3bp, \
                 tc.tile_pool(name="p3bc", bufs=2) as p3bc:
                carry2 = None
                for ch in range(NCH):
                    csl = slice(ch * CH, (ch + 1) * CH)
                    x1c = p3b.tile([128, NB, CH], f32r, tag="x1c")
                    nc.sync.dma_start(x1c[:], fm(x1T)[:, :, csl])
                    ln2 = p3b.tile([128, NB, CH], f32r, tag="ln2")
                    layernorm(p3b, p3bp, x1c, cf["ln2g"], cf["ln2b"], ln2)
                    xx0 = cmsh_sb[:, :, None] if ch == 0 else carry2
                    for nm, dram in (("cmk", xkfT), ("cmr", xrfT)):
                        ot = p3blo.tile([128, NB, CH], bf16, tag="lo2")
                        lerp_out(ln2, xx0, cf[nm], ot, dram, ch)
                    ncar = p3bc.tile([128, NB, 1], f32r, tag="carry2")
                    nc.vector.tensor_copy(ncar[:], ln2[:, :, CH - 1:CH])
                    carry2 = ncar
                    if ch == NCH - 1:
                        cms = p3bc.tile([128, NB, 1], f32, tag="cms")
                        nc.vector.tensor_copy(cms[:], ln2[:, :, CH - 1:CH])
                        nc.sync.dma_start(out_cmsh[:, :, None], cms[:])

            # =========================================================
            # P4: FFN k = relu(xk @ Wk_f.T)^2 -> ktf (bf16)
            # =========================================================
            with tc.tile_pool(name="p4a", bufs=1) as p4a, \
                 tc.tile_pool(name="p4w", bufs=4) as p4w, \
                 tc.tile_pool(name="p4o", bufs=4) as p4o, \
                 tc.tile_pool(name="p4p", bufs=2, space="PSUM") as p4p:
                for half in range(2):
                    tsl = slice(half * TH, (half + 1) * TH)
                    act = p4a.tile([128, NB, TH], bf16, tag="actf")
                    nc.sync.dma_start(
                        act[:], xkfT.rearrange("(b p) t -> p b t", p=128)[:, :, tsl])
                    for fb in range(NFB):
                        ps = [p4p.tile([128, CH], f32, tag=f"pf{i}", name=f"pf{i}")
                              for i in range(2)]
                        for ci in range(NB):
                            wblk = p4w.tile([128, 128], bf16, tag="wfblk")
                            nc.sync.dma_start(
                                wblk[:], WkfT[ci * 128:(ci + 1) * 128,
                                              fb * 128:(fb + 1) * 128])
                            for t2 in range(2):
                                nc.tensor.matmul(
                                    ps[t2][:], wblk[:],
                                    act[:, ci, t2 * CH:(t2 + 1) * CH],
                                    start=(ci == 0), stop=(ci == NB - 1))
                        for t2 in range(2):
                            rl = p4o.tile([128, CH], f32, tag="rl")
                            nc.vector.tensor_scalar_max(rl[:], ps[t2][:], 0.0)
                            kb = p4o.tile([128, CH], bf16, tag="kb")
                            nc.vector.tensor_mul(kb[:], ps[t2][:], rl[:])
                            nc.sync.dma_start(
                                ktf[fb * 128:(fb + 1) * 128,
                                    half * TH + t2 * CH:
                                    half * TH + (t2 + 1) * CH],
                                kb[:])

            # =========================================================
            # P5: kv + sigmoid gate + residual -> x2T
            # =========================================================
            with tc.tile_pool(name="p5v", bufs=1) as p5v, \
                 tc.tile_pool(name="p5k", bufs=4) as p5k, \
                 tc.tile_pool(name="p5", bufs=2) as p5, \
                 tc.tile_pool(name="p5g", bufs=8) as p5g, \
                 tc.tile_pool(name="p5p", bufs=1, space="PSUM") as p5p:
                for coh in range(2):
                    cosl = slice(coh * (C // 2), (coh + 1) * (C // 2))
                    wv_sb = p5v.tile([128, NFB, C // 2], bf16, tag="wv")
                    nc.sync.dma_start(
                        wv_sb[:], WvfT.rearrange("(fb p) c -> p fb c", p=128)
                        [:, :, cosl])
                    for tch in range(NCH):
                        tsl = slice(tch * CH, (tch + 1) * CH)
                        xrf = p5.tile([128, NB, CH], bf16, tag="xrf")
                        nc.sync.dma_start(
                            xrf[:], xrfT.rearrange("(b p) t -> p b t", p=128)
                            [:, :, tsl])
                        gss = []
                        for cb in range(8):
                            gp = p5p.tile([128, 8, CH], f32, tag="big")
                            for ci in range(NB):
                                wrblk = p5k.tile([128, 128], bf16, tag="wrblk")
                                nc.sync.dma_start(
                                    wrblk[:],
                                    WrfT[ci * 128:(ci + 1) * 128,
                                         (coh * 8 + cb) * 128:
                                         (coh * 8 + cb + 1) * 128])
                                nc.tensor.matmul(
                                    gp[:, 0, :], wrblk[:], xrf[:, ci, :],
                                    start=(ci == 0), stop=(ci == NB - 1))
                            gs = p5g.tile([128, CH], bf16, tag="gs")
                            nc.scalar.activation(gs[:], gp[:, 0, :], FP.Sigmoid)
                            gss.append(gs)
                        kvp = p5p.tile([128, 8, CH], f32, tag="big")
                        for fb in range(NFB):
                            kfb = p5k.tile([128, CH], bf16, tag="kfb")
                            nc.sync.dma_start(
                                kfb[:], ktf[fb * 128:(fb + 1) * 128, tsl])
                            for cb in range(8):
                                nc.tensor.matmul(
                                    kvp[:, cb, :],
                                    wv_sb[:, fb, cb * 128:(cb + 1) * 128],
                                    kfb[:],
                                    start=(fb == 0), stop=(fb == NFB - 1))
                        for cb in range(8):
                            co = coh * 8 + cb
                            x1b = p5.tile([128, CH], f32r, tag="x1b")
                            nc.sync.dma_start(
                                x1b[:], x1T[co * 128:(co + 1) * 128, tsl])
                            ffn = p5.tile([128, CH], f32, tag="ffn")
                            nc.vector.tensor_mul(ffn[:], kvp[:, cb, :], gss[cb][:])
                            nc.vector.tensor_add(ffn[:], ffn[:], x1b[:])
                            nc.sync.dma_start(
                                out_x2T[co * 128:(co + 1) * 128, tsl], ffn[:])

    nc.finalize()
    return nc


_cached_nc = None


def _get_nc():
    global _cached_nc
    if _cached_nc is None:
        _cached_nc = build_kernel()
    return _cached_nc


def _host_prep(inputs):
    f4 = np.float32
    dec = np.exp(-np.exp(np.asarray(inputs["decay"], np.float64)))
    u = np.asarray(inputs["faaaa"], f4)

    idx = np.arange(CH)
    d = idx[:, None] - idx[None, :]
    pw = dec[:, None, None] ** np.maximum(d - 1, 0)[None].astype(np.float64)
    wmat = np.where(d[None] > 0, pw,
                    np.where(d[None] == 0, u[:, None, None].astype(np.float64), 0.0))
    wmatT = np.ascontiguousarray(wmat.transpose(0, 2, 1)).astype(f4)

    wk = dec[:, None] ** (CH - 1 - idx)[None, :]
    wb = dec[:, None] ** idx[None, :]
    ws = (dec ** CH).astype(f4)

    tglob = np.arange(TT) % CH
    wk_full = wk[:, tglob].T.astype(f4)                       # [TT, H]
    wkt = np.ascontiguousarray(wk_full.reshape(NB, 128, H).transpose(1, 0, 2))
    wb_tiled = np.tile(wb.astype(f4), (1, NCH))          # [H, TT]
    wbT_full = np.ascontiguousarray(
        wb_tiled.reshape(NPAIR, 2, TT).transpose(1, 0, 2))  # [2, NPAIR, TT]

    ws_pair = np.empty((128, NPAIR), f4)
    for j in range(NPAIR):
        ws_pair[:64, j] = ws[2 * j]
        ws_pair[64:, j] = ws[2 * j + 1]

    mask2 = np.zeros((2, 128), f4)
    mask2[0, :64] = 1.0
    mask2[1, 64:] = 1.0
    seg2 = np.ascontiguousarray(mask2.T)

    def colmaj(v):
        return np.ascontiguousarray(np.asarray(v, f4).reshape(NB, 128).T)

    shared = dict(
        WrT=np.ascontiguousarray(np.asarray(inputs["Wr"], f4).T),
        WkT=np.ascontiguousarray(np.asarray(inputs["Wk"], f4).T),
        WvT=np.ascontiguousarray(np.asarray(inputs["Wv"], f4).T),
        WoT=np.ascontiguousarray(np.asarray(inputs["Wo"], f4).T),
        WkfT=np.ascontiguousarray(np.asarray(inputs["Wk_f"], f4).T).astype(ml_dtypes.bfloat16),
        WvfT=np.ascontiguousarray(np.asarray(inputs["Wv_f"], f4).T).astype(ml_dtypes.bfloat16),
        WrfT=np.ascontiguousarray(np.asarray(inputs["Wr_f"], f4).T).astype(ml_dtypes.bfloat16),
        wmatT=wmatT, wktok=wkt, wbT=wbT_full, wspair=ws_pair,
        tmk=colmaj(inputs["tmk"]), tmv=colmaj(inputs["tmv"]), tmr=colmaj(inputs["tmr"]),
        cmk=colmaj(inputs["cmk"]), cmr=colmaj(inputs["cmr"]),
        ln1g=colmaj(inputs["ln1_g"]), ln1b=colmaj(inputs["ln1_b"]),
        ln2g=colmaj(inputs["ln2_g"]), ln2b=colmaj(inputs["ln2_b"]),
        gng=colmaj(inputs["gn_g"]), gnb=colmaj(inputs["gn_b"]),
        mask2=mask2, seg2=seg2,
        ones_col=np.ones((128, 1), f4), ones_row=np.ones((1, 128), f4),
        ident=np.eye(128, dtype=f4),
    )

    in_maps = []
    for b in range(B):
        s0b = np.asarray(inputs["wkv_state"][b], f4)
        s0dev = np.empty((128, NPAIR, S), f4)
        for j in range(NPAIR):
            s0dev[:64, j] = s0b[2 * j]
            s0dev[64:, j] = s0b[2 * j + 1]
        m = dict(shared)
        m.update(
            xT=np.ascontiguousarray(np.asarray(inputs["x"][b], f4).T),
            tmshift=colmaj(inputs["tm_shift"][b]),
            cmshift=colmaj(inputs["cm_shift"][b]),
            s0=s0dev,
        )
        in_maps.append(m)
    return in_maps


def kernel(**inputs):
    nc = _get_nc()
    in_maps = _host_prep(inputs)
    res = run_bass_kernel_spmd(nc, in_maps, core_ids=list(range(B)), trace=False)
    x_out = np.empty((B, TT, C), np.float32)
    tm_sh = np.empty((B, C), np.float32)
    cm_sh = np.empty((B, C), np.float32)
    wkv = np.empty((B, H, S, S), np.float32)
    for b in range(B):
        r = res.results[b]
        x_out[b] = r["out_x2T"].T
        tm_sh[b] = r["out_tmsh"].T.reshape(C)
        cm_sh[b] = r["out_cmsh"].T.reshape(C)
        w = r["out_wkv"].reshape(2, 64, NPAIR, S)
        wkv[b] = np.ascontiguousarray(w.transpose(2, 0, 1, 3)).reshape(H, S, S)
    return x_out, tm_sh, wkv, cm_sh
